# revision 50
# baseline (speedup 1.0000x reference)
"""MeshGCN (6-layer GCN, N=100000, E=1200000) on 8 trn2 NeuronCores.

Strategy (graph/data parallel, batched SWDGE gather + scatter-add):
  - Nodes partitioned contiguously across 8 cores (12544 slots incl. pads);
    GCN weights replicated. Per layer: v = dinv*(x @ W) locally, AllGather the
    f32 v slabs into vfull [100352, 64] DRAM, then each core accumulates its
    ~150K in-edges: acc[dst] += vfull[src].
  - The accumulate is batched SWDGE work spread over 4 queues:
      * dma_gather: "windows" = global src-row ranges chosen so every core has
        <= WSLOT unique srcs and the range spans < 32768 rows (int16 indices
        with the range base). One gather call per window -> msg tile.
      * dma_scatter_add (SBUF dst, parity-split): per window, occurrence
        sub-calls o=0.. over slot prefixes K_o (slots sorted by multiplicity
        desc) scatter the o-th edge of each slot. Tokens are unique per
        sub-call (the CCE read-modify-write races on duplicates); conflicting
        edges recurse into later overflow levels. Accumulator pairs rotate so
        independent scatter calls pipeline.
  - The SAME tables serve all 6 layers (static graph). Last layer aggregates
    first and applies W5 after, so every gather is 64 f32 (256B descriptors).
"""
import os
import numpy as np

MINI = bool(int(os.environ.get("GCN_MINI", "0")))
if MINI:
    N, E, CHUNKS, WSLOT = 2040, int(os.environ.get("GCN_MINI_E", "8192")), 2, 128
else:
    N, E, CHUNKS, WSLOT = 100000, 1200000, 98, int(os.environ.get("GCN_WSLOT", "1792"))
NCORES = 8
P = 128
OWN = N // NCORES
OWNP = CHUNKS * P
CH = [8, 64, 64, 64, 64, 64, 3]
L = 6
F = 64
NROWS = NCORES * OWNP

NQ = int(os.environ.get("GCN_NQ", "4"))
NACC = int(os.environ.get("GCN_NACC", "4"))
NMSG = int(os.environ.get("GCN_NMSG", "4"))
SCRATCH = int(os.environ.get("GCN_SCRATCH", "16384"))
CAP = int(os.environ.get("GCN_CAP", "2"))      # max edges per gather slot (half)
AGSPLIT = bool(int(os.environ.get("GCN_AGSPLIT", "0")))  # 2-phase AllGather
PAIR = bool(int(os.environ.get("GCN_PAIR", "0")))  # gather bf16 node-pair rows
if PAIR and MINI:
    WSLOT = 128
elif PAIR:
    WSLOT = int(os.environ.get("GCN_WSLOT", "1280"))
TRASH = OWNP - 1

LAST_RESULTS = None

assert WSLOT % 128 == 0
assert (WSLOT // 16 + 1) * 2 <= SCRATCH // 64, "two calls per queue ring"


def _nrep():
    return int(os.environ.get("GCN_REPEAT",
                              "12" if os.environ.get("GCN_TIME") else "1"))


def _host_preprocess(edge_index):
    src = np.asarray(edge_index[0], dtype=np.int64)
    dst = np.asarray(edge_index[1], dtype=np.int64)

    deg_in = np.bincount(dst, minlength=N)
    dinv = (1.0 / np.sqrt(deg_in + 1.0)).astype(np.float32)

    # node local id n -> (p = n // CHUNKS, chunk = n % CHUNKS); vfull row of
    # global node g = core*OWNP + n  (publish layout: bounce row p*CHUNKS+c=n)
    g_core = src // OWN
    g_loc = src % OWN
    grow = g_core * OWNP + g_loc
    if AGSPLIT and not PAIR:
        # remapped row space: vfull1 holds every core's slab rows [0, HALF)
        # (= v partitions 0..63), vfull2 the rest; windows never cross the
        # boundary so each gather reads one table.
        HALF = OWNP // 2
        grow = np.where(
            g_loc < HALF,
            g_core * HALF + g_loc,
            NROWS // 2 + g_core * HALF + (g_loc - HALF),
        )
    d_core = dst // OWN
    d_loc = dst % OWN
    dtok = (d_loc // CHUNKS) + P * (d_loc % CHUNKS)

    # per-core edge lists sorted by src (pair-)row
    HN = 2 if PAIR else 1
    KEYROWS = NROWS // HN
    core_rows, core_half, core_toks = [], [], []
    for c in range(NCORES):
        m = d_core == c
        r, t = grow[m], dtok[m]
        o = np.argsort(r, kind="stable")
        r = r[o]
        core_rows.append(r >> (HN - 1))
        core_half.append((r & (HN - 1)).astype(np.int64))
        core_toks.append(t[o])

    # level/window construction with UNIFORM row boundaries across cores
    windows = []   # per window: base, hi, per-core slots (key rows) + edges
    pend = [(core_rows[c], core_half[c], core_toks[c]) for c in range(NCORES)]
    for _level in range(24):
        if not any(len(p[0]) for p in pend):
            break
        uniq = [np.unique(p[0]) for p in pend]
        pos = [0] * NCORES
        bounds = []
        b = 0
        while b < KEYROWS and any(pos[c] < len(uniq[c]) for c in range(NCORES)):
            nxt = min(b + 32768, KEYROWS)
            if AGSPLIT and not PAIR and b < KEYROWS // 2:
                nxt = min(nxt, KEYROWS // 2)
            for c in range(NCORES):
                u = uniq[c]
                while pos[c] < len(u) and u[pos[c]] < b:
                    pos[c] += 1
                weff = max(128, int(WSLOT * float(os.environ.get("GCN_WEFF", "0.9"))))
                if pos[c] + weff < len(u) and int(u[pos[c] + weff]) < nxt:
                    nxt = int(u[pos[c] + weff])
            for c in range(NCORES):
                u = uniq[c]
                while pos[c] < len(u) and u[pos[c]] < nxt:
                    pos[c] += 1
            bounds.append((b, nxt))
            b = nxt
        lvl_wins = []
        for (lo, hi) in bounds:
            lvl_wins.append({"base": lo, "hi": hi,
                             "slots": [None] * NCORES,
                             "edges": [None] * NCORES})
        nxt_pend = []
        for c in range(NCORES):
            rows, halfs, toks = pend[c]
            nr, nh, nt = [], [], []
            wi = 0
            slot_of = {}
            slots, edges, occ_tokens = [], [], []
            def flush(wi):
                lvl_wins[wi]["slots"][c] = slots
                lvl_wins[wi]["edges"][c] = edges
            for i in range(len(rows)):
                r, h, t = int(rows[i]), int(halfs[i]), int(toks[i])
                while wi < len(bounds) and r >= bounds[wi][1]:
                    flush(wi)
                    wi += 1
                    slot_of, slots, edges, occ_tokens = {}, [], [], []
                s = slot_of.get(r)
                if s is None or len(edges[s][h]) >= CAP:
                    if len(slots) >= WSLOT:
                        nr.append(r)
                        nh.append(h)
                        nt.append(t)
                        continue
                    s = len(slots)
                    slot_of[r] = s
                    slots.append(r)
                    edges.append([[] for _ in range(HN)])
                o = len(edges[s][h])
                if o < len(occ_tokens) and t in occ_tokens[o]:
                    nr.append(r)
                    nh.append(h)
                    nt.append(t)
                    continue
                while len(occ_tokens) <= o:
                    occ_tokens.append(set())
                occ_tokens[o].add(t)
                edges[s][h].append(t)
            while wi < len(bounds):
                flush(wi)
                wi += 1
                slot_of, slots, edges, occ_tokens = {}, [], [], []
            nxt_pend.append((np.asarray(nr, np.int64), np.asarray(nh, np.int64),
                             np.asarray(nt, np.int64)))
        windows.extend(lvl_wins)
        pend = nxt_pend
    assert not any(len(p[0]) for p in pend), "overflow recursion did not terminate"

    # per window: sort slots by max-half multiplicity desc, compute uniform K
    # (K counts NODE slots = HN per gather slot)
    NW = len(windows)
    KU = []
    for w in windows:
        nocc = 0
        for c in range(NCORES):
            slots = w["slots"][c] or []
            edges = w["edges"][c] or []
            mult = (np.array([max(len(hh) for hh in e) for e in edges], np.int64)
                    if edges else np.zeros(0, np.int64))
            perm = np.argsort(-mult, kind="stable")
            w["slots"][c] = [slots[j] for j in perm]
            w["edges"][c] = [edges[j] for j in perm]
            if len(edges):
                nocc = max(nocc, int(mult.max()))
        # scatter slot index of (pair position i, half h) in the DMA's
        # consumption order: s = i%128 + 128*(HN*(i//128) + h)
        K = []
        for o in range(nocc):
            k = 0
            for c in range(NCORES):
                for i, e2 in enumerate(w["edges"][c]):
                    for h in range(HN):
                        if len(e2[h]) > o:
                            s = (i % 128) + 128 * (HN * (i // 128) + h)
                            k = max(k, s + 1)
            K.append(min(-(-k // 128) * 128, HN * WSLOT))
        KU.append(K)

    # tables
    nslot_tot = NW * WSLOT
    ndesc_sc = sum(sum(K) for K in KU)
    gidx = np.zeros((NCORES, nslot_tot), np.int16)
    stok = np.full((NCORES, ndesc_sc), TRASH, np.int16)
    bases = np.zeros(NW, np.int64)
    his = np.zeros(NW, np.int64)
    off = 0
    for wi, w in enumerate(windows):
        bases[wi] = w["base"]
        his[wi] = w["hi"]
        for c in range(NCORES):
            slots = w["slots"][c]
            rel = np.asarray(slots, np.int64) - w["base"]
            assert rel.size == 0 or (rel.min() >= 0 and rel.max() < 32768)
            col = np.zeros(WSLOT, np.int16)
            col[:rel.size] = rel.astype(np.int16)
            gidx[c, wi * WSLOT:(wi + 1) * WSLOT] = col
        o0 = off
        for o, k in enumerate(KU[wi]):
            for c in range(NCORES):
                edges = w["edges"][c]
                for i, e2 in enumerate(edges):
                    for h in range(HN):
                        if o < len(e2[h]):
                            s = (i % 128) + 128 * (HN * (i // 128) + h)
                            assert s < k, (wi, o, i, h, s, k)
                            stok[c, o0 + s] = e2[h][o]
            o0 += k
        off = o0
    assert off == ndesc_sc

    def pack(v):
        n = v.shape[1]
        assert n % 16 == 0
        t = v.reshape(NCORES, n // 16, 16).transpose(0, 2, 1)
        return np.ascontiguousarray(np.tile(t, (1, 8, 1)))

    stats = {
        "NW": NW, "g_descs": nslot_tot, "s_descs": ndesc_sc,
        "calls": NW + sum(len(K) for K in KU),
    }
    return {
        "dinv": dinv, "gidx": pack(gidx), "stok": pack(stok),
        "bases": bases, "his": his, "NW": NW, "KU": KU,
        "nslot_tot": nslot_tot, "ndesc_sc": ndesc_sc, "stats": stats,
    }


def _patch_queue_aware_lanes():
    """Make TileClockTick assign DMASW sem lanes per SWDGE queue.

    The stock pass rotates the 8 DMASW lanes round-robin over all Pool DMA
    instructions regardless of queue_num, but the ucode locks each sem to the
    first queue that increments it -> queue q's instructions must always land
    on the same lane subset. Route queue q to lanes {q, q+NQ, q+2*NQ, ...}.
    """
    import concourse.tile_sem_assignment as tsa

    if getattr(tsa, "_gcn_queue_lanes", False):
        return
    tsa._gcn_queue_lanes = True
    orig = tsa.TileClockTick._assign_tick

    def patched(self, inst):
        eng = getattr(inst, "engine", None)
        from concourse import mybir as mb
        is_pool_dma = (
            isinstance(inst, tsa.DMAInst)
            and not isinstance(inst, tsa.bass_isa.UserSyncedRemoteDMADescs)
            and eng == mb.EngineType.Pool
            and not isinstance(inst, mb.InstCollectiveCompute)
        )
        if is_pool_dma:
            qn = int(getattr(inst, "queue_num", 0) or 0)
            tog = getattr(self, "_gcn_qtog", None)
            if tog is None:
                tog = {}
                self._gcn_qtog = tog
            nlane = max(1, self.swdge_sem_count // NQ)
            t = tog.get(qn, 0)
            tog[qn] = (t + 1) % nlane
            self.next_sw_dma_idx = qn + NQ * t
        return orig(self, inst)

    tsa.TileClockTick._assign_tick = patched


def _build_nc(bases, his, NW, KU):
    import concourse.bacc as bacc
    import concourse.bass as bass
    import concourse.tile as tile
    from concourse import mybir
    from concourse.masks import make_identity

    _patch_queue_aware_lanes()

    f32 = mybir.dt.float32
    i16 = mybir.dt.int16

    nc = bacc.Bacc(
        "TRN2", target_bir_lowering=False, debug=False,
        enable_asserts=True, num_devices=NCORES,
        dynamic_dma_scratch_size=SCRATCH, num_swdge_queues=NQ,
    )
    nslot_tot = NW * WSLOT
    ndesc_sc = sum(sum(K) for K in KU)
    CC = WSLOT // P
    HCH = CHUNKS // 2

    x0_d = nc.dram_tensor("x0", [P, CHUNKS * CH[0]], f32, kind="ExternalInput").ap()
    gi_d = nc.dram_tensor("gidx", [P, nslot_tot // 16], i16, kind="ExternalInput").ap()
    st_d = nc.dram_tensor("stok", [P, ndesc_sc // 16], i16, kind="ExternalInput").ap()
    dinv_d = nc.dram_tensor("dinvt", [P, CHUNKS], f32, kind="ExternalInput").ap()
    w_d = [nc.dram_tensor(f"w{l}", [CH[l], CH[l + 1]], f32, kind="ExternalInput").ap()
           for l in range(L)]
    b_d = [nc.dram_tensor(f"bt{l}", [P, CH[l + 1]], f32, kind="ExternalInput").ap()
           for l in range(L)]
    out_d = nc.dram_tensor("out", [P, CHUNKS * 3], f32, kind="ExternalOutput").ap()
    DBG = bool(os.environ.get("GCN_DBG"))
    if DBG:
        dbgv_d = nc.dram_tensor("dbgv", [NROWS, F], f32, kind="ExternalOutput").ap()
        dbgm_d = nc.dram_tensor("dbgm", [P, (2 if PAIR else 1) * (WSLOT // P) * F],
                                f32, kind="ExternalOutput").ap()
        dbga_d = nc.dram_tensor("dbga", [P, (CHUNKS // 2) * F], f32,
                                kind="ExternalOutput").ap()

    with tile.TileContext(nc) as tc:
        with (
            tc.tile_pool(name="const", bufs=1) as cp,
            tc.tile_pool(name="xv", bufs=2) as xvp,
            tc.tile_pool(name="small", bufs=3) as sp,
            tc.tile_pool(name="pt", bufs=3, space="PSUM") as ptp,
            tc.tile_pool(name="pm", bufs=3, space="PSUM") as pmp,
            tc.tile_pool(name="dramvf", bufs=2, space="DRAM") as dvp,
        ):
            gi_s = cp.tile([P, nslot_tot // 16], i16)
            nc.sync.dma_start(gi_s[:], gi_d[:])
            st_s = cp.tile([P, ndesc_sc // 16], i16)
            nc.sync.dma_start(st_s[:], st_d[:])
            dinv_s = cp.tile([P, CHUNKS], f32)
            nc.sync.dma_start(dinv_s[:], dinv_d[:])
            w_s, b_s = [], []
            for l in range(L):
                w = cp.tile([CH[l], CH[l + 1]], f32, tag=f"w{l}")
                nc.sync.dma_start(w[:], w_d[l][:])
                w_s.append(w)
                b = cp.tile([P, CH[l + 1]], f32, tag=f"b{l}")
                nc.sync.dma_start(b[:], b_d[l][:])
                b_s.append(b)
            ident = cp.tile([P, P], f32)
            make_identity(nc, ident[:])
            x0_s = cp.tile([P, CHUNKS * CH[0]], f32)
            nc.sync.dma_start(x0_s[:], x0_d[:])

            HN = 2 if PAIR else 1
            msg = cp.tile([P, NMSG * HN * CC * F], f32)
            if PAIR:
                bf16_t = mybir.dt.bfloat16
                pbuf = cp.tile([P, NMSG * CC * 2 * F], bf16_t)
            accs = []
            for k in range(NACC):
                ae = cp.tile([P, HCH * F], f32, tag=f"ae{k}")
                ao = cp.tile([P, HCH * F], f32, tag=f"ao{k}")
                accs.append((ae, ao))

            bf16 = mybir.dt.bfloat16
            AGBF = bool(os.environ.get("GCN_AGBF"))
            vbounce = dvp.tile([OWNP, F], bf16 if (AGBF or PAIR) else f32,
                               tag="vb")
            if DBG:
                dbga_s_tile = cp.tile([P, (CHUNKS // 2) * F], f32, tag="dbga")
            vb_view = vbounce[:].rearrange("(p c) f -> p (c f)", p=P)
            SPLIT = AGSPLIT and not PAIR and not AGBF
            if SPLIT:
                HALF = OWNP // 2
                vb1 = dvp.tile([HALF, F], f32, tag="vb1")
                vb2 = dvp.tile([HALF, F], f32, tag="vb2")
                vb1_view = vb1[:].rearrange("(p c) f -> p (c f)", p=P // 2)
                vb2_view = vb2[:].rearrange("(p c) f -> p (c f)", p=P // 2)

            x_cur = None
            call_n = [0]

            def nextq():
                q = call_n[0] % NQ
                call_n[0] += 1
                return q

            nrep = _nrep()
            for l in [ll for _ in range(nrep) for ll in range(L)]:
                fin = CH[l]
                v = xvp.tile([P, CHUNKS * F], f32, tag="xv")
                if l < L - 1:
                    xin = x0_s if l == 0 else x_cur
                    for c in range(CHUNKS):
                        pt = ptp.tile([F, P], f32, tag="tp", space="PSUM")
                        nc.tensor.transpose(
                            out=pt[:fin, :], in_=xin[:, c * fin:(c + 1) * fin],
                            identity=ident[:],
                        )
                        xT = sp.tile([F, P], f32, tag="xT")
                        nc.vector.tensor_copy(out=xT[:fin, :], in_=pt[:fin, :])
                        pu = pmp.tile([P, F], f32, tag="mm", space="PSUM")
                        nc.tensor.matmul(
                            out=pu[:], lhsT=xT[:fin, :], rhs=w_s[l][:],
                            start=True, stop=True,
                        )
                        nc.vector.tensor_scalar(
                            out=v[:, c * F:(c + 1) * F], in0=pu[:],
                            scalar1=dinv_s[:, c:c + 1], scalar2=None,
                            op0=mybir.AluOpType.mult,
                        )
                else:
                    v3 = v[:].rearrange("p (c f) -> p c f", f=F)
                    x3 = x_cur[:].rearrange("p (c f) -> p c f", f=F)
                    nc.vector.tensor_tensor(
                        out=v3, in0=x3,
                        in1=dinv_s[:].to_broadcast([P, CHUNKS, F]),
                        op=mybir.AluOpType.mult,
                    )

                if PAIR:
                    vfull_bf = dvp.tile([NROWS, F], bf16, tag="vfullbf",
                                        addr_space="Shared")
                    nc.gpsimd.dma_start(vb_view, v[:])   # f32 -> bf16 cast
                    if not os.environ.get("GCN_NOAG"):
                        nc.gpsimd.collective_compute(
                            "AllGather", mybir.AluOpType.bypass,
                            replica_groups=[list(range(NCORES))],
                            ins=[vbounce.opt()], outs=[vfull_bf.opt()],
                        )
                    vpair = vfull_bf[:].rearrange("(q two) f -> q (two f)", two=2)
                elif AGBF:
                    vfull = dvp.tile([NROWS, F], f32, tag="vfull",
                                     addr_space="Shared")
                    vfull_bf = dvp.tile([NROWS, F], bf16, tag="vfullbf",
                                        addr_space="Shared")
                    nc.gpsimd.dma_start(vb_view, v[:])   # f32 -> bf16 cast
                    if not os.environ.get("GCN_NOAG"):
                        nc.gpsimd.collective_compute(
                            "AllGather", mybir.AluOpType.bypass,
                            replica_groups=[list(range(NCORES))],
                            ins=[vbounce.opt()], outs=[vfull_bf.opt()],
                        )
                    nc.gpsimd.dma_start(vfull[:], vfull_bf[:])  # bf16 -> f32
                elif SPLIT:
                    vfull1 = dvp.tile([NROWS // 2, F], f32, tag="vfull1",
                                      addr_space="Shared")
                    vfull2 = dvp.tile([NROWS // 2, F], f32, tag="vfull2",
                                      addr_space="Shared")
                    nc.sync.dma_start(vb1_view, v[: P // 2, :])
                    nc.sync.dma_start(vb2_view, v[P // 2:, :])
                    if not os.environ.get("GCN_NOAG"):
                        nc.gpsimd.collective_compute(
                            "AllGather", mybir.AluOpType.bypass,
                            replica_groups=[list(range(NCORES))],
                            ins=[vb1.opt()], outs=[vfull1.opt()],
                        )
                        nc.gpsimd.collective_compute(
                            "AllGather", mybir.AluOpType.bypass,
                            replica_groups=[list(range(NCORES))],
                            ins=[vb2.opt()], outs=[vfull2.opt()],
                        )
                else:
                    vfull = dvp.tile([NROWS, F], f32, tag="vfull",
                                     addr_space="Shared")
                    nc.sync.dma_start(vb_view, v[:])
                    if not os.environ.get("GCN_NOAG"):
                        nc.gpsimd.collective_compute(
                            "AllGather", mybir.AluOpType.bypass,
                            replica_groups=[list(range(NCORES))],
                            ins=[vbounce.opt()], outs=[vfull.opt()],
                        )

                for (ae, ao) in accs:
                    nc.vector.memset(ae[:], 0.0)
                    nc.vector.memset(ao[:], 0.0)

                sc_pos = 0
                acc_i = 0
                QDED = bool(int(os.environ.get("GCN_QDED", "1")))
                for w in range(NW):
                    h = w % NMSG
                    mv = msg[:, h * HN * CC * F:(h + 1) * HN * CC * F]
                    m3 = mv.rearrange("p (c f) -> p c f", f=F)
                    lo = int(bases[w])
                    if PAIR:
                        hi = min(lo + 32768, NROWS // 2)
                        pv = pbuf[:, h * CC * 2 * F:(h + 1) * CC * 2 * F]
                        p3 = pv.rearrange("p (c f) -> p c f", f=2 * F)
                        nc.gpsimd.dma_gather(
                            p3, vpair[lo:hi, :],
                            gi_s[:, w * (WSLOT // 16):(w + 1) * (WSLOT // 16)],
                            WSLOT, WSLOT, 2 * F,
                            single_packet=False, queue_num=nextq(),
                        )
                        nc.vector.tensor_copy(out=mv, in_=pv)
                    elif SPLIT:
                        if lo < NROWS // 2:
                            vt, lo2 = vfull1, lo
                        else:
                            vt, lo2 = vfull2, lo - NROWS // 2
                        hi2 = min(lo2 + 32768, NROWS // 2)
                        nc.gpsimd.dma_gather(
                            m3, vt[:][lo2:hi2, :],
                            gi_s[:, w * (WSLOT // 16):(w + 1) * (WSLOT // 16)],
                            WSLOT, WSLOT, F,
                            single_packet=False, queue_num=nextq(),
                        )
                    else:
                        hi = min(lo + 32768, NROWS)
                        nc.gpsimd.dma_gather(
                            m3, vfull[:][lo:hi, :],
                            gi_s[:, w * (WSLOT // 16):(w + 1) * (WSLOT // 16)],
                            WSLOT, WSLOT, F,
                            single_packet=False,
                            queue_num=((w % 2) * 2 if QDED else nextq()),
                        )
                    if DBG and l == 0 and w == 0:
                        nc.sync.dma_start(dbgm_d[:], mv)
                    for o, k in enumerate(KU[w]):
                        ae, ao = accs[(w + o) % NACC]
                        acc_i += 1
                        iv = m3[:, : k // P, :]
                        nc.gpsimd.dma_scatter_add(
                            ae[:], iv,
                            st_s[:, sc_pos // 16:(sc_pos + k) // 16],
                            k, k, F,
                            single_packet=False,
                            queue_num=(1 + 2 * (acc_i % 2) if QDED else nextq()),
                            sbuf_tokens_per_rank=128, parity_reg=0,
                            out_ap_other=ao[:],
                        )
                        sc_pos += k
                if DBG and l == 0:
                    if PAIR:
                        nc.gpsimd.dma_start(dbgv_d[:], vfull_bf[:])
                    elif SPLIT:
                        nc.gpsimd.dma_start(dbgv_d[:][: NROWS // 2, :], vfull1[:])
                        nc.gpsimd.dma_start(dbgv_d[:][NROWS // 2:, :], vfull2[:])
                    else:
                        nc.gpsimd.dma_start(dbgv_d[:], vfull[:])
                    nc.vector.tensor_copy(out=dbga_s_tile[:], in_=accs[0][0][:])
                    nc.sync.dma_start(dbga_d[:], dbga_s_tile[:])

                # fold: v = dinv * (v + sum accs); parity layout on accs
                ae0, ao0 = accs[0]
                for (ae, ao) in accs[1:]:
                    nc.vector.tensor_tensor(out=ae0[:], in0=ae0[:], in1=ae[:],
                                            op=mybir.AluOpType.add)
                    nc.vector.tensor_tensor(out=ao0[:], in0=ao0[:], in1=ao[:],
                                            op=mybir.AluOpType.add)
                v3 = v[:].rearrange("p (c f) -> p c f", f=F)
                vap = v[:]
                for par, acc in ((0, ae0), (1, ao0)):
                    a3 = acc[:].rearrange("p (g f) -> p g f", f=F)
                    rv = bass.AP(vap.tensor, vap.offset + par * F,
                                 [vap.ap[0], [2 * F, HCH], [1, F]])
                    nc.vector.tensor_tensor(out=rv, in0=rv, in1=a3,
                                            op=mybir.AluOpType.add)
                nc.vector.tensor_tensor(
                    out=v3, in0=v3,
                    in1=dinv_s[:].to_broadcast([P, CHUNKS, F]),
                    op=mybir.AluOpType.mult,
                )
                if l < L - 1:
                    bap = b_s[l][:]
                    bb = bass.AP(bap.tensor, bap.offset,
                                 [bap.ap[0], [0, CHUNKS], bap.ap[1]])
                    nc.vector.tensor_tensor(
                        out=v3, in0=v3, in1=bb, op=mybir.AluOpType.add,
                    )
                    nc.scalar.activation(
                        out=v[:], in_=v[:],
                        func=mybir.ActivationFunctionType.Relu,
                    )
                    x_cur = v
                else:
                    outs = cp.tile([P, CHUNKS * 3], f32)
                    for c in range(CHUNKS):
                        pt = ptp.tile([F, P], f32, tag="tp", space="PSUM")
                        nc.tensor.transpose(
                            out=pt[:], in_=v[:, c * F:(c + 1) * F],
                            identity=ident[:],
                        )
                        zT = sp.tile([F, P], f32, tag="xT")
                        nc.vector.tensor_copy(out=zT[:], in_=pt[:])
                        po = pmp.tile([P, F], f32, tag="mm", space="PSUM")
                        nc.tensor.matmul(
                            out=po[:, :3], lhsT=zT[:], rhs=w_s[L - 1][:],
                            start=True, stop=True,
                        )
                        nc.vector.tensor_tensor(
                            out=outs[:, c * 3:(c + 1) * 3], in0=po[:, :3],
                            in1=b_s[L - 1][:], op=mybir.AluOpType.add,
                        )
                    nc.sync.dma_start(out_d[:], outs[:])

    nc.compile()
    return nc


def _run_timed(nc, in_maps, iters=24):
    import time
    import jax
    import numpy as np
    from jax.sharding import Mesh, PartitionSpec, NamedSharding
    from jax.experimental.shard_map import shard_map
    from concourse import bass2jax, mybir

    bass2jax.install_neuronx_cc_hook()
    partition_name = nc.partition_id_tensor.name if nc.partition_id_tensor else None
    in_names, out_names, out_avals, zero_outs = [], [], [], []
    for alloc in nc.m.functions[0].allocations:
        if not isinstance(alloc, mybir.MemoryLocationSet):
            continue
        name = alloc.memorylocations[0].name
        if alloc.kind == "ExternalInput":
            if name != partition_name:
                in_names.append(name)
        elif alloc.kind == "ExternalOutput":
            shape = tuple(alloc.tensor_shape)
            dtype = mybir.dt.np(alloc.dtype)
            out_names.append(name)
            out_avals.append(jax.core.ShapedArray(shape, dtype))
            zero_outs.append(np.zeros(shape, dtype))
    n_params = len(in_names)
    all_names = in_names + out_names
    if partition_name is not None:
        all_names = all_names + [partition_name]

    def _body(*args):
        operands = list(args)
        if partition_name is not None:
            operands.append(bass2jax.partition_id_tensor())
        outs = bass2jax._bass_exec_p.bind(
            *operands,
            out_avals=tuple(out_avals),
            in_names=tuple(all_names),
            out_names=tuple(out_names),
            lowering_input_output_aliases=(),
            sim_require_finite=True,
            sim_require_nnan=True,
            nc=nc,
        )
        return tuple(outs)

    devices = jax.devices()[:NCORES]
    mesh = Mesh(np.asarray(devices), ("core",))
    spec = PartitionSpec("core")
    n_outs = len(zero_outs)
    nin = n_params + n_outs
    donate = tuple(range(n_params, nin))
    fn = jax.jit(
        shard_map(_body, mesh=mesh, in_specs=(spec,) * nin,
                  out_specs=(spec,) * len(out_names), check_rep=False),
        donate_argnums=donate, keep_unused=True,
    )
    concat_in = [
        np.concatenate([np.asarray(in_maps[c][name]) for c in range(NCORES)], axis=0)
        for name in in_names
    ]
    concat_zeros = [
        np.zeros((NCORES * z.shape[0], *z.shape[1:]), z.dtype) for z in zero_outs
    ]
    sh = NamedSharding(mesh, spec)
    dev_in = [jax.device_put(a, sh) for a in concat_in]

    def _zeros():
        return [jax.device_put(z, sh) for z in concat_zeros]

    outs = fn(*dev_in, *_zeros())
    jax.block_until_ready(outs)
    zs_list = [_zeros() for _ in range(iters)]
    for zs in zs_list:
        jax.block_until_ready(zs)
    t0 = time.perf_counter()
    all_outs = [fn(*dev_in, *zs) for zs in zs_list]
    jax.block_until_ready(all_outs)
    dt = time.perf_counter() - t0
    best_ns = int(dt / iters * 1e9 / _nrep())
    results = [
        {name: np.asarray(all_outs[-1][i]).reshape(NCORES, *out_avals[i].shape)[c]
         for i, name in enumerate(out_names)}
        for c in range(NCORES)
    ]
    print(f"[timed] per-iter s: {dt / iters:.4f}")
    return type("R", (), {"results": results, "exec_time_ns": best_ns})()


def kernel(x, edge_index, W0, b0, W1, b1, W2, b2, W3, b3, W4, b4, W5, b5):
    global LAST_RESULTS
    from concourse import bass_utils

    x = np.asarray(x, dtype=np.float32)
    Ws = [np.asarray(w, dtype=np.float32) for w in (W0, W1, W2, W3, W4, W5)]
    bs = [np.asarray(b, dtype=np.float32) for b in (b0, b1, b2, b3, b4, b5)]

    pre = _host_preprocess(np.asarray(edge_index))
    dinv = pre["dinv"]
    if os.environ.get("GCN_STATS"):
        print("[sched]", pre["stats"])

    nc = _build_nc(pre["bases"], pre["his"], pre["NW"], pre["KU"])

    in_maps = []
    for c in range(NCORES):
        nl = np.arange(OWN)
        p_ = nl // CHUNKS
        c_ = nl % CHUNKS
        x0 = np.zeros((P, CHUNKS, CH[0]), np.float32)
        dv = np.zeros((P, CHUNKS), np.float32)
        x0[p_, c_] = x[c * OWN + nl]
        dv[p_, c_] = dinv[c * OWN + nl]
        m = {
            "x0": x0.reshape(P, CHUNKS * CH[0]),
            "gidx": pre["gidx"][c],
            "stok": pre["stok"][c],
            "dinvt": dv,
        }
        for l in range(L):
            m[f"w{l}"] = Ws[l]
            m[f"bt{l}"] = np.broadcast_to(bs[l], (P, CH[l + 1])).copy()
        in_maps.append(m)

    if os.environ.get("GCN_RUN", "hw") == "sim":
        from concourse.bass_interp import MultiCoreSim
        sim = MultiCoreSim(nc, num_cores=NCORES)
        for c in range(NCORES):
            for name, arr in in_maps[c].items():
                sim.cores[c].tensor(name)[:] = arr
        sim.simulate()
        onames = ["out"] + (["dbgv", "dbgm", "dbga"]
                            if os.environ.get("GCN_DBG") else [])
        results = [{n: np.array(sim.cores[c].tensor(n)) for n in onames}
                   for c in range(NCORES)]
        res = type("R", (), {"results": results, "exec_time_ns": None})()
    elif os.environ.get("GCN_TIME"):
        res = _run_timed(nc, in_maps)
    else:
        res = bass_utils.run_bass_kernel_spmd(
            nc, in_maps, core_ids=list(range(NCORES)),
        )
    LAST_RESULTS = res

    out = np.zeros((N, 3), dtype=np.float32)
    for c in range(NCORES):
        slab = res.results[c]["out"].reshape(P, CHUNKS, 3)
        nl = np.arange(OWN)
        out[c * OWN + nl] = slab[nl // CHUNKS, nl % CHUNKS]
    return out


# revision 51
# speedup vs baseline: 1.2056x; 1.2056x over previous
"""MeshGCN (6-layer GCN, N=100000, E=1200000) on 8 trn2 NeuronCores.

Strategy (graph/data parallel, batched SWDGE gather + scatter-add):
  - Nodes partitioned contiguously across 8 cores (12544 slots incl. pads);
    GCN weights replicated. Per layer: v = dinv*(x @ W) locally, AllGather the
    f32 v slabs into vfull [100352, 64] DRAM, then each core accumulates its
    ~150K in-edges: acc[dst] += vfull[src].
  - The accumulate is batched SWDGE work spread over 4 queues:
      * dma_gather: "windows" = global src-row ranges chosen so every core has
        <= WSLOT unique srcs and the range spans < 32768 rows (int16 indices
        with the range base). One gather call per window -> msg tile.
      * dma_scatter_add (SBUF dst, parity-split): per window, occurrence
        sub-calls o=0.. over slot prefixes K_o (slots sorted by multiplicity
        desc) scatter the o-th edge of each slot. Tokens are unique per
        sub-call (the CCE read-modify-write races on duplicates); conflicting
        edges recurse into later overflow levels. Accumulator pairs rotate so
        independent scatter calls pipeline.
  - The SAME tables serve all 6 layers (static graph). Last layer aggregates
    first and applies W5 after, so every gather is 64 f32 (256B descriptors).
"""
import os
import numpy as np

MINI = bool(int(os.environ.get("GCN_MINI", "0")))
if MINI:
    N, E, CHUNKS, WSLOT = 2040, int(os.environ.get("GCN_MINI_E", "8192")), 2, 128
else:
    N, E, CHUNKS, WSLOT = 100000, 1200000, 98, int(os.environ.get("GCN_WSLOT", "1792"))
NCORES = 8
P = 128
OWN = N // NCORES
OWNP = CHUNKS * P
CH = [8, 64, 64, 64, 64, 64, 3]
L = 6
F = 64
NROWS = NCORES * OWNP

NQ = int(os.environ.get("GCN_NQ", "4"))
NACC = int(os.environ.get("GCN_NACC", "4"))
NMSG = int(os.environ.get("GCN_NMSG", "4"))
SCRATCH = int(os.environ.get("GCN_SCRATCH", "16384"))
CAP = int(os.environ.get("GCN_CAP", "2"))      # max edges per gather slot (half)
AGSPLIT = bool(int(os.environ.get("GCN_AGSPLIT", "0")))  # 2-phase AllGather
PAIR = bool(int(os.environ.get("GCN_PAIR", "0")))  # gather bf16 node-pair rows
if PAIR and MINI:
    WSLOT = 128
elif PAIR:
    WSLOT = int(os.environ.get("GCN_WSLOT", "1280"))
TRASH = OWNP - 1

LAST_RESULTS = None

assert WSLOT % 128 == 0
assert (WSLOT // 16 + 1) * 2 <= SCRATCH // 64, "two calls per queue ring"


def _nrep():
    return int(os.environ.get("GCN_REPEAT",
                              "12" if os.environ.get("GCN_TIME") else "1"))


def _host_preprocess(edge_index):
    src = np.asarray(edge_index[0], dtype=np.int64)
    dst = np.asarray(edge_index[1], dtype=np.int64)

    deg_in = np.bincount(dst, minlength=N)
    dinv = (1.0 / np.sqrt(deg_in + 1.0)).astype(np.float32)

    # node local id n -> (p = n // CHUNKS, chunk = n % CHUNKS); vfull row of
    # global node g = core*OWNP + n  (publish layout: bounce row p*CHUNKS+c=n)
    g_core = src // OWN
    g_loc = src % OWN
    grow = g_core * OWNP + g_loc
    if AGSPLIT and not PAIR:
        # remapped row space: vfull1 holds every core's slab rows [0, HALF)
        # (= v partitions 0..63), vfull2 the rest; windows never cross the
        # boundary so each gather reads one table.
        HALF = OWNP // 2
        grow = np.where(
            g_loc < HALF,
            g_core * HALF + g_loc,
            NROWS // 2 + g_core * HALF + (g_loc - HALF),
        )
    d_core = dst // OWN
    d_loc = dst % OWN
    dtok = (d_loc // CHUNKS) + P * (d_loc % CHUNKS)

    # per-core edge lists sorted by src (pair-)row
    HN = 2 if PAIR else 1
    KEYROWS = NROWS // HN
    core_rows, core_half, core_toks = [], [], []
    for c in range(NCORES):
        m = d_core == c
        r, t = grow[m], dtok[m]
        o = np.argsort(r, kind="stable")
        r = r[o]
        core_rows.append(r >> (HN - 1))
        core_half.append((r & (HN - 1)).astype(np.int64))
        core_toks.append(t[o])

    # level/window construction with UNIFORM row boundaries across cores
    windows = []   # per window: base, hi, per-core slots (key rows) + edges
    pend = [(core_rows[c], core_half[c], core_toks[c]) for c in range(NCORES)]
    for _level in range(24):
        if not any(len(p[0]) for p in pend):
            break
        uniq = [np.unique(p[0]) for p in pend]
        pos = [0] * NCORES
        bounds = []
        b = 0
        while b < KEYROWS and any(pos[c] < len(uniq[c]) for c in range(NCORES)):
            nxt = min(b + 32768, KEYROWS)
            if AGSPLIT and not PAIR and b < KEYROWS // 2:
                nxt = min(nxt, KEYROWS // 2)
            for c in range(NCORES):
                u = uniq[c]
                while pos[c] < len(u) and u[pos[c]] < b:
                    pos[c] += 1
                weff = max(128, int(WSLOT * float(os.environ.get("GCN_WEFF", "0.9"))))
                if pos[c] + weff < len(u) and int(u[pos[c] + weff]) < nxt:
                    nxt = int(u[pos[c] + weff])
            for c in range(NCORES):
                u = uniq[c]
                while pos[c] < len(u) and u[pos[c]] < nxt:
                    pos[c] += 1
            bounds.append((b, nxt))
            b = nxt
        lvl_wins = []
        for (lo, hi) in bounds:
            lvl_wins.append({"base": lo, "hi": hi,
                             "slots": [None] * NCORES,
                             "edges": [None] * NCORES})
        nxt_pend = []
        for c in range(NCORES):
            rows, halfs, toks = pend[c]
            nr, nh, nt = [], [], []
            wi = 0
            slot_of = {}
            slots, edges, occ_tokens = [], [], []
            def flush(wi):
                lvl_wins[wi]["slots"][c] = slots
                lvl_wins[wi]["edges"][c] = edges
            for i in range(len(rows)):
                r, h, t = int(rows[i]), int(halfs[i]), int(toks[i])
                while wi < len(bounds) and r >= bounds[wi][1]:
                    flush(wi)
                    wi += 1
                    slot_of, slots, edges, occ_tokens = {}, [], [], []
                s = slot_of.get(r)
                if s is None or len(edges[s][h]) >= CAP:
                    if len(slots) >= WSLOT:
                        nr.append(r)
                        nh.append(h)
                        nt.append(t)
                        continue
                    s = len(slots)
                    slot_of[r] = s
                    slots.append(r)
                    edges.append([[] for _ in range(HN)])
                o = len(edges[s][h])
                if o < len(occ_tokens) and t in occ_tokens[o]:
                    nr.append(r)
                    nh.append(h)
                    nt.append(t)
                    continue
                while len(occ_tokens) <= o:
                    occ_tokens.append(set())
                occ_tokens[o].add(t)
                edges[s][h].append(t)
            while wi < len(bounds):
                flush(wi)
                wi += 1
                slot_of, slots, edges, occ_tokens = {}, [], [], []
            nxt_pend.append((np.asarray(nr, np.int64), np.asarray(nh, np.int64),
                             np.asarray(nt, np.int64)))
        windows.extend(lvl_wins)
        pend = nxt_pend
    assert not any(len(p[0]) for p in pend), "overflow recursion did not terminate"

    # per window: sort slots by max-half multiplicity desc, compute uniform K
    # (K counts NODE slots = HN per gather slot)
    NW = len(windows)
    KU = []
    for w in windows:
        nocc = 0
        for c in range(NCORES):
            slots = w["slots"][c] or []
            edges = w["edges"][c] or []
            mult = (np.array([max(len(hh) for hh in e) for e in edges], np.int64)
                    if edges else np.zeros(0, np.int64))
            perm = np.argsort(-mult, kind="stable")
            w["slots"][c] = [slots[j] for j in perm]
            w["edges"][c] = [edges[j] for j in perm]
            if len(edges):
                nocc = max(nocc, int(mult.max()))
        # scatter slot index of (pair position i, half h) in the DMA's
        # consumption order: s = i%128 + 128*(HN*(i//128) + h)
        K = []
        for o in range(nocc):
            k = 0
            for c in range(NCORES):
                for i, e2 in enumerate(w["edges"][c]):
                    for h in range(HN):
                        if len(e2[h]) > o:
                            s = (i % 128) + 128 * (HN * (i // 128) + h)
                            k = max(k, s + 1)
            K.append(min(-(-k // 128) * 128, HN * WSLOT))
        KU.append(K)

    # tables
    nslot_tot = NW * WSLOT
    ndesc_sc = sum(sum(K) for K in KU)
    gidx = np.zeros((NCORES, nslot_tot), np.int16)
    stok = np.full((NCORES, ndesc_sc), TRASH, np.int16)
    bases = np.zeros(NW, np.int64)
    his = np.zeros(NW, np.int64)
    off = 0
    for wi, w in enumerate(windows):
        bases[wi] = w["base"]
        his[wi] = w["hi"]
        for c in range(NCORES):
            slots = w["slots"][c]
            rel = np.asarray(slots, np.int64) - w["base"]
            assert rel.size == 0 or (rel.min() >= 0 and rel.max() < 32768)
            col = np.zeros(WSLOT, np.int16)
            col[:rel.size] = rel.astype(np.int16)
            gidx[c, wi * WSLOT:(wi + 1) * WSLOT] = col
        o0 = off
        for o, k in enumerate(KU[wi]):
            for c in range(NCORES):
                edges = w["edges"][c]
                for i, e2 in enumerate(edges):
                    for h in range(HN):
                        if o < len(e2[h]):
                            s = (i % 128) + 128 * (HN * (i // 128) + h)
                            assert s < k, (wi, o, i, h, s, k)
                            stok[c, o0 + s] = e2[h][o]
            o0 += k
        off = o0
    assert off == ndesc_sc

    def pack(v):
        n = v.shape[1]
        assert n % 16 == 0
        t = v.reshape(NCORES, n // 16, 16).transpose(0, 2, 1)
        return np.ascontiguousarray(np.tile(t, (1, 8, 1)))

    stats = {
        "NW": NW, "g_descs": nslot_tot, "s_descs": ndesc_sc,
        "calls": NW + sum(len(K) for K in KU),
    }
    return {
        "dinv": dinv, "gidx": pack(gidx), "stok": pack(stok),
        "bases": bases, "his": his, "NW": NW, "KU": KU,
        "nslot_tot": nslot_tot, "ndesc_sc": ndesc_sc, "stats": stats,
    }


def _patch_queue_aware_lanes():
    """Make TileClockTick assign DMASW sem lanes per SWDGE queue.

    The stock pass rotates the 8 DMASW lanes round-robin over all Pool DMA
    instructions regardless of queue_num, but the ucode locks each sem to the
    first queue that increments it -> queue q's instructions must always land
    on the same lane subset. Route queue q to lanes {q, q+NQ, q+2*NQ, ...}.
    """
    import concourse.tile_sem_assignment as tsa

    if getattr(tsa, "_gcn_queue_lanes", False):
        return
    tsa._gcn_queue_lanes = True
    orig = tsa.TileClockTick._assign_tick

    def patched(self, inst):
        eng = getattr(inst, "engine", None)
        from concourse import mybir as mb
        is_pool_dma = (
            isinstance(inst, tsa.DMAInst)
            and not isinstance(inst, tsa.bass_isa.UserSyncedRemoteDMADescs)
            and eng == mb.EngineType.Pool
            and not isinstance(inst, mb.InstCollectiveCompute)
        )
        if is_pool_dma:
            qn = int(getattr(inst, "queue_num", 0) or 0)
            tog = getattr(self, "_gcn_qtog", None)
            if tog is None:
                tog = {}
                self._gcn_qtog = tog
            nlane = max(1, self.swdge_sem_count // NQ)
            t = tog.get(qn, 0)
            tog[qn] = (t + 1) % nlane
            self.next_sw_dma_idx = qn + NQ * t
        return orig(self, inst)

    tsa.TileClockTick._assign_tick = patched


def _build_nc(bases, his, NW, KU):
    import concourse.bacc as bacc
    import concourse.bass as bass
    import concourse.tile as tile
    from concourse import mybir
    from concourse.masks import make_identity

    _patch_queue_aware_lanes()

    f32 = mybir.dt.float32
    i16 = mybir.dt.int16

    nc = bacc.Bacc(
        "TRN2", target_bir_lowering=False, debug=False,
        enable_asserts=True, num_devices=NCORES,
        dynamic_dma_scratch_size=SCRATCH, num_swdge_queues=NQ,
    )
    nslot_tot = NW * WSLOT
    ndesc_sc = sum(sum(K) for K in KU)
    CC = WSLOT // P
    HCH = CHUNKS // 2

    x0_d = nc.dram_tensor("x0", [P, CHUNKS * CH[0]], f32, kind="ExternalInput").ap()
    gi_d = nc.dram_tensor("gidx", [P, nslot_tot // 16], i16, kind="ExternalInput").ap()
    st_d = nc.dram_tensor("stok", [P, ndesc_sc // 16], i16, kind="ExternalInput").ap()
    dinv_d = nc.dram_tensor("dinvt", [P, CHUNKS], f32, kind="ExternalInput").ap()
    w_d = [nc.dram_tensor(f"w{l}", [CH[l], CH[l + 1]], f32, kind="ExternalInput").ap()
           for l in range(L)]
    b_d = [nc.dram_tensor(f"bt{l}", [P, CH[l + 1]], f32, kind="ExternalInput").ap()
           for l in range(L)]
    out_d = nc.dram_tensor("out", [P, CHUNKS * 3], f32, kind="ExternalOutput").ap()
    DBG = bool(os.environ.get("GCN_DBG"))
    if DBG:
        dbgv_d = nc.dram_tensor("dbgv", [NROWS, F], f32, kind="ExternalOutput").ap()
        dbgm_d = nc.dram_tensor("dbgm", [P, (2 if PAIR else 1) * (WSLOT // P) * F],
                                f32, kind="ExternalOutput").ap()
        dbga_d = nc.dram_tensor("dbga", [P, (CHUNKS // 2) * F], f32,
                                kind="ExternalOutput").ap()

    with tile.TileContext(nc) as tc:
        with (
            tc.tile_pool(name="const", bufs=1) as cp,
            tc.tile_pool(name="xv", bufs=2) as xvp,
            tc.tile_pool(name="small", bufs=3) as sp,
            tc.tile_pool(name="pt", bufs=3, space="PSUM") as ptp,
            tc.tile_pool(name="pm", bufs=3, space="PSUM") as pmp,
            tc.tile_pool(name="dramvf", bufs=2, space="DRAM") as dvp,
        ):
            gi_s = cp.tile([P, nslot_tot // 16], i16)
            nc.sync.dma_start(gi_s[:], gi_d[:])
            st_s = cp.tile([P, ndesc_sc // 16], i16)
            nc.sync.dma_start(st_s[:], st_d[:])
            dinv_s = cp.tile([P, CHUNKS], f32)
            nc.sync.dma_start(dinv_s[:], dinv_d[:])
            w_s, b_s = [], []
            for l in range(L):
                w = cp.tile([CH[l], CH[l + 1]], f32, tag=f"w{l}")
                nc.sync.dma_start(w[:], w_d[l][:])
                w_s.append(w)
                b = cp.tile([P, CH[l + 1]], f32, tag=f"b{l}")
                nc.sync.dma_start(b[:], b_d[l][:])
                b_s.append(b)
            ident = cp.tile([P, P], f32)
            make_identity(nc, ident[:])
            x0_s = cp.tile([P, CHUNKS * CH[0]], f32)
            nc.sync.dma_start(x0_s[:], x0_d[:])

            HN = 2 if PAIR else 1
            msg = cp.tile([P, NMSG * HN * CC * F], f32)
            if PAIR:
                bf16_t = mybir.dt.bfloat16
                pbuf = cp.tile([P, NMSG * CC * 2 * F], bf16_t)
            accs = []
            for k in range(NACC):
                ae = cp.tile([P, HCH * F], f32, tag=f"ae{k}")
                ao = cp.tile([P, HCH * F], f32, tag=f"ao{k}")
                accs.append((ae, ao))

            bf16 = mybir.dt.bfloat16
            AGBF = bool(os.environ.get("GCN_AGBF"))
            vbounce = dvp.tile([OWNP, F], bf16 if (AGBF or PAIR) else f32,
                               tag="vb")
            if DBG:
                dbga_s_tile = cp.tile([P, (CHUNKS // 2) * F], f32, tag="dbga")
            vb_view = vbounce[:].rearrange("(p c) f -> p (c f)", p=P)
            SPLIT = AGSPLIT and not PAIR and not AGBF
            if SPLIT:
                HALF = OWNP // 2
                vb1 = dvp.tile([HALF, F], f32, tag="vb1")
                vb2 = dvp.tile([HALF, F], f32, tag="vb2")
                vb1_view = vb1[:].rearrange("(p c) f -> p (c f)", p=P // 2)
                vb2_view = vb2[:].rearrange("(p c) f -> p (c f)", p=P // 2)

            x_cur = None
            call_n = [0]

            def nextq():
                q = call_n[0] % NQ
                call_n[0] += 1
                return q

            nrep = _nrep()
            for l in [ll for _ in range(nrep) for ll in range(L)]:
                fin = CH[l]
                v = xvp.tile([P, CHUNKS * F], f32, tag="xv")
                if l < L - 1:
                    xin = x0_s if l == 0 else x_cur
                    for c in range(CHUNKS):
                        pt = ptp.tile([F, P], f32, tag="tp", space="PSUM")
                        nc.tensor.transpose(
                            out=pt[:fin, :], in_=xin[:, c * fin:(c + 1) * fin],
                            identity=ident[:],
                        )
                        xT = sp.tile([F, P], f32, tag="xT")
                        nc.vector.tensor_copy(out=xT[:fin, :], in_=pt[:fin, :])
                        pu = pmp.tile([P, F], f32, tag="mm", space="PSUM")
                        nc.tensor.matmul(
                            out=pu[:], lhsT=xT[:fin, :], rhs=w_s[l][:],
                            start=True, stop=True,
                        )
                        nc.vector.tensor_scalar(
                            out=v[:, c * F:(c + 1) * F], in0=pu[:],
                            scalar1=dinv_s[:, c:c + 1], scalar2=None,
                            op0=mybir.AluOpType.mult,
                        )
                else:
                    v3 = v[:].rearrange("p (c f) -> p c f", f=F)
                    x3 = x_cur[:].rearrange("p (c f) -> p c f", f=F)
                    nc.vector.tensor_tensor(
                        out=v3, in0=x3,
                        in1=dinv_s[:].to_broadcast([P, CHUNKS, F]),
                        op=mybir.AluOpType.mult,
                    )

                if PAIR:
                    vfull_bf = dvp.tile([NROWS, F], bf16, tag="vfullbf",
                                        addr_space="Shared")
                    nc.gpsimd.dma_start(vb_view, v[:])   # f32 -> bf16 cast
                    if not os.environ.get("GCN_NOAG"):
                        nc.gpsimd.collective_compute(
                            "AllGather", mybir.AluOpType.bypass,
                            replica_groups=[list(range(NCORES))],
                            ins=[vbounce.opt()], outs=[vfull_bf.opt()],
                        )
                    vpair = vfull_bf[:].rearrange("(q two) f -> q (two f)", two=2)
                elif AGBF:
                    vfull = dvp.tile([NROWS, F], f32, tag="vfull",
                                     addr_space="Shared")
                    vfull_bf = dvp.tile([NROWS, F], bf16, tag="vfullbf",
                                        addr_space="Shared")
                    nc.gpsimd.dma_start(vb_view, v[:])   # f32 -> bf16 cast
                    if not os.environ.get("GCN_NOAG"):
                        nc.gpsimd.collective_compute(
                            "AllGather", mybir.AluOpType.bypass,
                            replica_groups=[list(range(NCORES))],
                            ins=[vbounce.opt()], outs=[vfull_bf.opt()],
                        )
                    nc.gpsimd.dma_start(vfull[:], vfull_bf[:])  # bf16 -> f32
                elif SPLIT:
                    vfull1 = dvp.tile([NROWS // 2, F], f32, tag="vfull1",
                                      addr_space="Shared")
                    vfull2 = dvp.tile([NROWS // 2, F], f32, tag="vfull2",
                                      addr_space="Shared")
                    nc.sync.dma_start(vb1_view, v[: P // 2, :])
                    nc.sync.dma_start(vb2_view, v[P // 2:, :])
                    if not os.environ.get("GCN_NOAG"):
                        nc.gpsimd.collective_compute(
                            "AllGather", mybir.AluOpType.bypass,
                            replica_groups=[list(range(NCORES))],
                            ins=[vb1.opt()], outs=[vfull1.opt()],
                        )
                        nc.gpsimd.collective_compute(
                            "AllGather", mybir.AluOpType.bypass,
                            replica_groups=[list(range(NCORES))],
                            ins=[vb2.opt()], outs=[vfull2.opt()],
                        )
                else:
                    vfull = dvp.tile([NROWS, F], f32, tag="vfull",
                                     addr_space="Shared")
                    nc.sync.dma_start(vb_view, v[:])
                    if not os.environ.get("GCN_NOAG"):
                        nc.gpsimd.collective_compute(
                            "AllGather", mybir.AluOpType.bypass,
                            replica_groups=[list(range(NCORES))],
                            ins=[vbounce.opt()], outs=[vfull.opt()],
                        )

                for (ae, ao) in accs:
                    nc.vector.memset(ae[:], 0.0)
                    nc.vector.memset(ao[:], 0.0)

                sc_pos = 0
                acc_i = 0
                QDED = bool(int(os.environ.get("GCN_QDED", "1")))
                for w in range(NW):
                    h = w % NMSG
                    mv = msg[:, h * HN * CC * F:(h + 1) * HN * CC * F]
                    m3 = mv.rearrange("p (c f) -> p c f", f=F)
                    lo = int(bases[w])
                    if PAIR:
                        hi = min(lo + 32768, NROWS // 2)
                        pv = pbuf[:, h * CC * 2 * F:(h + 1) * CC * 2 * F]
                        p3 = pv.rearrange("p (c f) -> p c f", f=2 * F)
                        nc.gpsimd.dma_gather(
                            p3, vpair[lo:hi, :],
                            gi_s[:, w * (WSLOT // 16):(w + 1) * (WSLOT // 16)],
                            WSLOT, WSLOT, 2 * F,
                            single_packet=False, queue_num=nextq(),
                        )
                        nc.vector.tensor_copy(out=mv, in_=pv)
                    elif SPLIT:
                        if lo < NROWS // 2:
                            vt, lo2 = vfull1, lo
                        else:
                            vt, lo2 = vfull2, lo - NROWS // 2
                        hi2 = min(lo2 + 32768, NROWS // 2)
                        nc.gpsimd.dma_gather(
                            m3, vt[:][lo2:hi2, :],
                            gi_s[:, w * (WSLOT // 16):(w + 1) * (WSLOT // 16)],
                            WSLOT, WSLOT, F,
                            single_packet=False, queue_num=nextq(),
                        )
                    else:
                        hi = min(lo + 32768, NROWS)
                        nc.gpsimd.dma_gather(
                            m3, vfull[:][lo:hi, :],
                            gi_s[:, w * (WSLOT // 16):(w + 1) * (WSLOT // 16)],
                            WSLOT, WSLOT, F,
                            single_packet=False,
                            queue_num=((w % 2) * 2 if QDED else nextq()),
                        )
                    if DBG and l == 0 and w == 0:
                        nc.sync.dma_start(dbgm_d[:], mv)
                    for o, k in enumerate(KU[w]):
                        ae, ao = accs[acc_i % NACC]
                        acc_i += 1
                        iv = m3[:, : k // P, :]
                        nc.gpsimd.dma_scatter_add(
                            ae[:], iv,
                            st_s[:, sc_pos // 16:(sc_pos + k) // 16],
                            k, k, F,
                            single_packet=False,
                            queue_num=(1 + 2 * (acc_i % 2) if QDED else nextq()),
                            sbuf_tokens_per_rank=128, parity_reg=0,
                            out_ap_other=ao[:],
                        )
                        sc_pos += k
                if DBG and l == 0:
                    if PAIR:
                        nc.gpsimd.dma_start(dbgv_d[:], vfull_bf[:])
                    elif SPLIT:
                        nc.gpsimd.dma_start(dbgv_d[:][: NROWS // 2, :], vfull1[:])
                        nc.gpsimd.dma_start(dbgv_d[:][NROWS // 2:, :], vfull2[:])
                    else:
                        nc.gpsimd.dma_start(dbgv_d[:], vfull[:])
                    nc.vector.tensor_copy(out=dbga_s_tile[:], in_=accs[0][0][:])
                    nc.sync.dma_start(dbga_d[:], dbga_s_tile[:])

                # fold: v = dinv * (v + sum accs); parity layout on accs
                ae0, ao0 = accs[0]
                for (ae, ao) in accs[1:]:
                    nc.vector.tensor_tensor(out=ae0[:], in0=ae0[:], in1=ae[:],
                                            op=mybir.AluOpType.add)
                    nc.vector.tensor_tensor(out=ao0[:], in0=ao0[:], in1=ao[:],
                                            op=mybir.AluOpType.add)
                v3 = v[:].rearrange("p (c f) -> p c f", f=F)
                vap = v[:]
                for par, acc in ((0, ae0), (1, ao0)):
                    a3 = acc[:].rearrange("p (g f) -> p g f", f=F)
                    rv = bass.AP(vap.tensor, vap.offset + par * F,
                                 [vap.ap[0], [2 * F, HCH], [1, F]])
                    nc.vector.tensor_tensor(out=rv, in0=rv, in1=a3,
                                            op=mybir.AluOpType.add)
                nc.vector.tensor_tensor(
                    out=v3, in0=v3,
                    in1=dinv_s[:].to_broadcast([P, CHUNKS, F]),
                    op=mybir.AluOpType.mult,
                )
                if l < L - 1:
                    bap = b_s[l][:]
                    bb = bass.AP(bap.tensor, bap.offset,
                                 [bap.ap[0], [0, CHUNKS], bap.ap[1]])
                    nc.vector.tensor_tensor(
                        out=v3, in0=v3, in1=bb, op=mybir.AluOpType.add,
                    )
                    nc.scalar.activation(
                        out=v[:], in_=v[:],
                        func=mybir.ActivationFunctionType.Relu,
                    )
                    x_cur = v
                else:
                    outs = cp.tile([P, CHUNKS * 3], f32)
                    for c in range(CHUNKS):
                        pt = ptp.tile([F, P], f32, tag="tp", space="PSUM")
                        nc.tensor.transpose(
                            out=pt[:], in_=v[:, c * F:(c + 1) * F],
                            identity=ident[:],
                        )
                        zT = sp.tile([F, P], f32, tag="xT")
                        nc.vector.tensor_copy(out=zT[:], in_=pt[:])
                        po = pmp.tile([P, F], f32, tag="mm", space="PSUM")
                        nc.tensor.matmul(
                            out=po[:, :3], lhsT=zT[:], rhs=w_s[L - 1][:],
                            start=True, stop=True,
                        )
                        nc.vector.tensor_tensor(
                            out=outs[:, c * 3:(c + 1) * 3], in0=po[:, :3],
                            in1=b_s[L - 1][:], op=mybir.AluOpType.add,
                        )
                    nc.sync.dma_start(out_d[:], outs[:])

    nc.compile()
    return nc


def _run_timed(nc, in_maps, iters=24):
    import time
    import jax
    import numpy as np
    from jax.sharding import Mesh, PartitionSpec, NamedSharding
    from jax.experimental.shard_map import shard_map
    from concourse import bass2jax, mybir

    bass2jax.install_neuronx_cc_hook()
    partition_name = nc.partition_id_tensor.name if nc.partition_id_tensor else None
    in_names, out_names, out_avals, zero_outs = [], [], [], []
    for alloc in nc.m.functions[0].allocations:
        if not isinstance(alloc, mybir.MemoryLocationSet):
            continue
        name = alloc.memorylocations[0].name
        if alloc.kind == "ExternalInput":
            if name != partition_name:
                in_names.append(name)
        elif alloc.kind == "ExternalOutput":
            shape = tuple(alloc.tensor_shape)
            dtype = mybir.dt.np(alloc.dtype)
            out_names.append(name)
            out_avals.append(jax.core.ShapedArray(shape, dtype))
            zero_outs.append(np.zeros(shape, dtype))
    n_params = len(in_names)
    all_names = in_names + out_names
    if partition_name is not None:
        all_names = all_names + [partition_name]

    def _body(*args):
        operands = list(args)
        if partition_name is not None:
            operands.append(bass2jax.partition_id_tensor())
        outs = bass2jax._bass_exec_p.bind(
            *operands,
            out_avals=tuple(out_avals),
            in_names=tuple(all_names),
            out_names=tuple(out_names),
            lowering_input_output_aliases=(),
            sim_require_finite=True,
            sim_require_nnan=True,
            nc=nc,
        )
        return tuple(outs)

    devices = jax.devices()[:NCORES]
    mesh = Mesh(np.asarray(devices), ("core",))
    spec = PartitionSpec("core")
    n_outs = len(zero_outs)
    nin = n_params + n_outs
    donate = tuple(range(n_params, nin))
    fn = jax.jit(
        shard_map(_body, mesh=mesh, in_specs=(spec,) * nin,
                  out_specs=(spec,) * len(out_names), check_rep=False),
        donate_argnums=donate, keep_unused=True,
    )
    concat_in = [
        np.concatenate([np.asarray(in_maps[c][name]) for c in range(NCORES)], axis=0)
        for name in in_names
    ]
    concat_zeros = [
        np.zeros((NCORES * z.shape[0], *z.shape[1:]), z.dtype) for z in zero_outs
    ]
    sh = NamedSharding(mesh, spec)
    dev_in = [jax.device_put(a, sh) for a in concat_in]

    def _zeros():
        return [jax.device_put(z, sh) for z in concat_zeros]

    outs = fn(*dev_in, *_zeros())
    jax.block_until_ready(outs)
    zs_list = [_zeros() for _ in range(iters)]
    for zs in zs_list:
        jax.block_until_ready(zs)
    t0 = time.perf_counter()
    all_outs = [fn(*dev_in, *zs) for zs in zs_list]
    jax.block_until_ready(all_outs)
    dt = time.perf_counter() - t0
    best_ns = int(dt / iters * 1e9 / _nrep())
    results = [
        {name: np.asarray(all_outs[-1][i]).reshape(NCORES, *out_avals[i].shape)[c]
         for i, name in enumerate(out_names)}
        for c in range(NCORES)
    ]
    print(f"[timed] per-iter s: {dt / iters:.4f}")
    return type("R", (), {"results": results, "exec_time_ns": best_ns})()


def kernel(x, edge_index, W0, b0, W1, b1, W2, b2, W3, b3, W4, b4, W5, b5):
    global LAST_RESULTS
    from concourse import bass_utils

    x = np.asarray(x, dtype=np.float32)
    Ws = [np.asarray(w, dtype=np.float32) for w in (W0, W1, W2, W3, W4, W5)]
    bs = [np.asarray(b, dtype=np.float32) for b in (b0, b1, b2, b3, b4, b5)]

    pre = _host_preprocess(np.asarray(edge_index))
    dinv = pre["dinv"]
    if os.environ.get("GCN_STATS"):
        print("[sched]", pre["stats"])

    nc = _build_nc(pre["bases"], pre["his"], pre["NW"], pre["KU"])

    in_maps = []
    for c in range(NCORES):
        nl = np.arange(OWN)
        p_ = nl // CHUNKS
        c_ = nl % CHUNKS
        x0 = np.zeros((P, CHUNKS, CH[0]), np.float32)
        dv = np.zeros((P, CHUNKS), np.float32)
        x0[p_, c_] = x[c * OWN + nl]
        dv[p_, c_] = dinv[c * OWN + nl]
        m = {
            "x0": x0.reshape(P, CHUNKS * CH[0]),
            "gidx": pre["gidx"][c],
            "stok": pre["stok"][c],
            "dinvt": dv,
        }
        for l in range(L):
            m[f"w{l}"] = Ws[l]
            m[f"bt{l}"] = np.broadcast_to(bs[l], (P, CH[l + 1])).copy()
        in_maps.append(m)

    if os.environ.get("GCN_RUN", "hw") == "sim":
        from concourse.bass_interp import MultiCoreSim
        sim = MultiCoreSim(nc, num_cores=NCORES)
        for c in range(NCORES):
            for name, arr in in_maps[c].items():
                sim.cores[c].tensor(name)[:] = arr
        sim.simulate()
        onames = ["out"] + (["dbgv", "dbgm", "dbga"]
                            if os.environ.get("GCN_DBG") else [])
        results = [{n: np.array(sim.cores[c].tensor(n)) for n in onames}
                   for c in range(NCORES)]
        res = type("R", (), {"results": results, "exec_time_ns": None})()
    elif os.environ.get("GCN_TIME"):
        res = _run_timed(nc, in_maps)
    else:
        res = bass_utils.run_bass_kernel_spmd(
            nc, in_maps, core_ids=list(range(NCORES)),
        )
    LAST_RESULTS = res

    out = np.zeros((N, 3), dtype=np.float32)
    for c in range(NCORES):
        slab = res.results[c]["out"].reshape(P, CHUNKS, 3)
        nl = np.arange(OWN)
        out[c * OWN + nl] = slab[nl // CHUNKS, nl % CHUNKS]
    return out
